# revision 49
# baseline (speedup 1.0000x reference)
"""Trainium2 Bass kernel for nn_AttentionMaskGenerator.

Math (verified against the reference):
  z[b,s,t] = x[b,s,:] @ W[t,:] + bias[t] - ln(-ln(u[b,s,t] + 1e-10) + 1e-10)
  tt[b,s]  = argmax_t z  (softmax + straight-through chain == plain argmax)
  row s of the [S,S] mask is:
    - tt == 1 : only the diagonal element (forward window ∩ causal == diag)
    - else    : full causal prefix  (next_global > s always, so the "local"
                constraint never binds under causal)
  output = broadcast over the 8 KV heads -> [B, 8, S, S] f32.

Sharding: data-parallel over (batch x head-pairs). Core c handles batch
c // 4 and emits that batch's [S,S] mask twice (head pair 2*(c%4), 2*(c%4)+1).
Each core writes only the non-zero (lower-trapezoid) columns of each
128-row block; ExternalOutput DRAM is pre-zeroed by the runtime (both the
native run_neff path and the bass2jax/PJRT path), so the strictly-upper
triangle is never written. Row blocks are processed in DESCENDING order:
the wide stores go out first, keeping the DMA wire saturated while the
cheap remaining flag chunks compute.

Engine split: DVE computes the logits (one mul+reduce per row block) and
the small compare/fix-up ops, ACT does the Ln chain up front and half of
the per-row broadcasts (DVE the other half), GPSIMD does the triangle
trim via affine_select, and the two HWDGE rings (sync + scalar queues)
each stream one head's output stores. The PE is left empty on purpose —
its instruction-stream fetch at kernel start costs ~1us per IRAM block
and sits on the critical path.
"""

import numpy as np

B, S, D, T, H = 2, 2048, 128, 3, 8
P = 128           # partitions / row-block size
NB = S // P       # 16 row blocks
CB = 4            # row blocks per phase-A chunk
EPS = 1e-10
N_CORES = 8

_CACHE = {}


def _build_program():
    import concourse.bass as bass
    import concourse.bacc as bacc
    import concourse.tile as tile
    from concourse import mybir
    from contextlib import ExitStack

    f32 = mybir.dt.float32
    Alu = mybir.AluOpType
    Act = mybir.ActivationFunctionType

    nc = bacc.Bacc("TRN2", debug=False, num_devices=N_CORES)
    x_ap = nc.dram_tensor("x", [S, D], f32, kind="ExternalInput").ap()
    u_ap = nc.dram_tensor("u", [S, T], f32, kind="ExternalInput").ap()
    w_ap = nc.dram_tensor("w", [T, D], f32, kind="ExternalInput").ap()
    bb_ap = nc.dram_tensor("bb", [T], f32, kind="ExternalInput").ap()
    out_ap = nc.dram_tensor("out", [2, S, S], f32, kind="ExternalOutput").ap()

    # process row blocks in DESCENDING order: late blocks carry the widest
    # stores, so issuing them first keeps the DMA wire saturated while the
    # remaining (cheap) flag chunks compute; the first chunk is small to
    # minimize time-to-first-store.
    chunk_list = [(14, 2), (12, 2), (10, 2), (8, 2), (4, 4), (0, 4)]

    with tile.TileContext(nc) as tc, ExitStack() as ctx:
        singles = ctx.enter_context(tc.tile_pool(name="singles", bufs=1))
        outp = ctx.enter_context(tc.tile_pool(name="outp", bufs=NB))
        chunks = ctx.enter_context(tc.tile_pool(name="chunks", bufs=2))

        # --- input loads (sync/HWDGE queue — it is idle until stores begin) ---
        # W broadcast to every partition: wb[p, t, d] = W[t, d]; first in the
        # queue because the first logits mul is gated on its completion
        wb = singles.tile([P, T, D], f32)
        nc.sync.dma_start(out=wb, in_=bass.AP(w_ap.tensor, 0, [[0, P], [D, T], [1, D]]))
        # x_all[p, i, d] = x[128*i + p, d]; loaded in chunk order so the first
        # chunk's slice (and its completion semaphore) lands first
        x_all = singles.tile([P, NB, D], f32)
        u_all = singles.tile([P, NB, T], f32)
        for n, (c0, cb) in enumerate(chunk_list):
            nc.sync.dma_start(
                out=x_all[:, c0 : c0 + cb, :],
                in_=bass.AP(
                    x_ap.tensor, c0 * P * D, [[D, P], [P * D, cb], [1, D]]
                ),
            )
            if n == 1:
                # u_all[p, i, t] = u[128*i + p, t]
                nc.sync.dma_start(
                    out=u_all, in_=bass.AP(u_ap.tensor, 0, [[T, P], [P * T, NB], [1, T]])
                )

        # --- constants (gpsimd queue, off the critical store queue) ---
        # bias broadcast: bbb[p, t] = bias[t]
        bbb = singles.tile([P, T], f32)
        nc.gpsimd.dma_start(out=bbb, in_=bass.AP(bb_ap.tensor, 0, [[0, P], [1, T]]))
        zeros = singles.tile([P, S], f32)
        nc.vector.memset(zeros, 0.0)
        eps_t = singles.tile([P, 1], f32)
        nc.vector.memset(eps_t, EPS)
        ones128 = singles.tile([P, P], f32)
        nc.gpsimd.memset(ones128, 1.0)
        # diag128[p, j] = 1.0 if j == p else 0.0
        diag128 = singles.tile([P, P], f32)
        nc.gpsimd.affine_select(
            out=diag128, in_=ones128, pattern=[[-1, P]],
            compare_op=Alu.is_equal, fill=0.0, base=0, channel_multiplier=1,
        )

        nf = singles.tile([P, NB], f32)
        nfa = nf[:]
        ba = bbb[:]

        # gumbel for ALL blocks up front: g2 = ln(-ln(u + eps) + eps); done
        # once on ACT so no per-chunk critical chain crosses engines twice
        g1 = singles.tile([P, NB, T], f32)
        nc.scalar.activation(g1[:], u_all[:], Act.Ln, bias=eps_t[:, 0:1], scale=1.0)
        g2 = singles.tile([P, NB, T], f32)
        nc.scalar.activation(g2[:], g1[:], Act.Ln, bias=eps_t[:, 0:1], scale=-1.0)

        for c0, cb in chunk_list:
            # --- phase A for blocks [c0, c0+cb): logits on DVE, one block per
            # mul/reduce so no single op clogs the DVE queue for long ---
            prod = chunks.tile([P, T, D], f32, tag="prod")
            logits = chunks.tile([P, CB, T], f32, tag="logits")
            wa = wb[:]
            for j in range(cb):
                # prod[p, t, d] = x_all[p, c0+j, d] * wb[p, t, d]
                xa = x_all[:, c0 + j, :]
                x_b = bass.AP(xa.tensor, xa.offset, [xa.ap[0], [0, T], xa.ap[1]])
                nc.vector.tensor_mul(prod[:], x_b, wa)
                nc.vector.reduce_sum(
                    logits[:, j, :], prod[:], axis=mybir.AxisListType.X
                )

            # z = logits + g + bias
            z = chunks.tile([P, CB, T], f32, tag="z")
            nc.vector.tensor_sub(z[:, 0:cb], logits[:, 0:cb], g2[:, c0 : c0 + cb, :])
            b_b = bass.AP(ba.tensor, ba.offset, [ba.ap[0], [0, cb], ba.ap[1]])
            nc.vector.tensor_add(z[:, 0:cb], z[:, 0:cb], b_b)

            # notflag[p, i] = (z1 <= z0) | (z1 < z2)  == !(argmax picks index 1)
            za = z[:]

            def zcol(t):
                return bass.AP(za.tensor, za.offset + t, [za.ap[0], [T, cb]])

            c1 = chunks.tile([P, CB], f32, tag="c1")
            nc.vector.tensor_tensor(c1[:, 0:cb], zcol(1), zcol(0), op=Alu.is_le)
            c2 = chunks.tile([P, CB], f32, tag="c2")
            nc.vector.tensor_tensor(c2[:, 0:cb], zcol(1), zcol(2), op=Alu.is_lt)
            nc.vector.tensor_max(nfa[:, c0 : c0 + cb], c1[:, 0:cb], c2[:, 0:cb])

            # --- phase B for blocks [c0, c0+cb), widest stores first ---
            for i in range(c0 + cb - 1, c0 - 1, -1):
                r0 = P * i
                w_cols = r0 + P
                nf_col = nfa[:, i : i + 1]

                ot = outp.tile([P, w_cols], f32, tag=f"ot{i}", bufs=1)
                split = i >= 14  # first blocks out: overlap bcast with stores
                if r0 > 0 and split:
                    # first half of the prefix broadcast, shipped immediately
                    # so the wire starts while the second half builds
                    half = r0 // 2
                    nc.scalar.activation(
                        ot[:, 0:half], zeros[:, 0:half], Act.Identity,
                        bias=nf_col, scale=1.0,
                    )
                    nc.sync.dma_start(
                        out=out_ap[0, r0 : r0 + P, 0:half], in_=ot[:, 0:half]
                    )
                    nc.scalar.dma_start(
                        out=out_ap[1, r0 : r0 + P, 0:half], in_=ot[:, 0:half]
                    )
                    nc.scalar.activation(
                        ot[:, half:r0], zeros[:, half:r0], Act.Identity,
                        bias=nf_col, scale=1.0,
                    )
                elif r0 > 0:
                    # prefix cols [0, r0): notflag broadcast (ACT only — DVE
                    # broadcasts sit in the DVE queue between chunk A-chains
                    # and delay the next blocks' flags)
                    nc.scalar.activation(
                        ot[:, 0:r0], zeros[:, 0:r0], Act.Identity,
                        bias=nf_col, scale=1.0,
                    )
                # diag chunk: notflag broadcast, trimmed to lower triangle,
                # then the diagonal forced to 1 (covers tt==1 rows) — both on
                # GPSIMD so the store waits on a single engine chain
                nf_bcast = bass.AP(nfa.tensor, nfa.offset + i, [nfa.ap[0], [0, P]])
                nc.gpsimd.affine_select(
                    out=ot[:, r0:w_cols], in_=nf_bcast, pattern=[[-1, P]],
                    compare_op=Alu.is_ge, fill=0.0, base=0, channel_multiplier=1,
                )
                nc.gpsimd.affine_select(
                    out=ot[:, r0:w_cols], in_=ot[:, r0:w_cols], pattern=[[-1, P]],
                    compare_op=Alu.not_equal, fill=1.0, base=0, channel_multiplier=1,
                )

                # one head per HWDGE ring (sync + scalar) -> 2x the DMA lanes
                lo = r0 // 2 if split else 0
                nc.sync.dma_start(
                    out=out_ap[0, r0 : r0 + P, lo:w_cols], in_=ot[:, lo:w_cols]
                )
                nc.scalar.dma_start(
                    out=out_ap[1, r0 : r0 + P, lo:w_cols], in_=ot[:, lo:w_cols]
                )

    nc.compile()
    return nc


def _get_program():
    if "nc" not in _CACHE:
        _CACHE["nc"] = _build_program()
    return _CACHE["nc"]


def _make_in_maps(input_tensor, gumbel_u, W, b):
    x = np.ascontiguousarray(np.asarray(input_tensor, dtype=np.float32))
    u = np.ascontiguousarray(np.asarray(gumbel_u, dtype=np.float32))
    w = np.ascontiguousarray(np.asarray(W, dtype=np.float32))
    bb = np.ascontiguousarray(np.asarray(b, dtype=np.float32))
    in_maps = []
    for c in range(N_CORES):
        bi = c // (N_CORES // B)
        in_maps.append({"x": x[bi], "u": u[bi], "w": w, "bb": bb})
    return in_maps


def _assemble(results):
    full = np.empty((B, H, S, S), dtype=np.float32)
    for c in range(N_CORES):
        bi = c // (N_CORES // B)
        q = c % (N_CORES // B)
        full[bi, 2 * q] = results[c]["out"][0]
        full[bi, 2 * q + 1] = results[c]["out"][1]
    return full


def kernel(input_tensor, token_types, gumbel_u, W, b, **_ignored):
    from concourse.bass_utils import run_bass_kernel_spmd

    nc = _get_program()
    in_maps = _make_in_maps(input_tensor, gumbel_u, W, b)
    res = run_bass_kernel_spmd(nc, in_maps, core_ids=list(range(N_CORES)))
    return _assemble(res.results)


# revision 51
# speedup vs baseline: 1.0261x; 1.0261x over previous
"""Trainium2 Bass kernel for nn_AttentionMaskGenerator.

Math (verified against the reference):
  z[b,s,t] = x[b,s,:] @ W[t,:] + bias[t] - ln(-ln(u[b,s,t] + 1e-10) + 1e-10)
  tt[b,s]  = argmax_t z  (softmax + straight-through chain == plain argmax)
  row s of the [S,S] mask is:
    - tt == 1 : only the diagonal element (forward window ∩ causal == diag)
    - else    : full causal prefix  (next_global > s always, so the "local"
                constraint never binds under causal)
  output = broadcast over the 8 KV heads -> [B, 8, S, S] f32.

Sharding: data-parallel over (batch x head-pairs). Core c handles batch
c // 4 and emits that batch's [S,S] mask twice (head pair 2*(c%4), 2*(c%4)+1).
Each core writes only the non-zero (lower-trapezoid) columns of each
128-row block; ExternalOutput DRAM is pre-zeroed by the runtime (both the
native run_neff path and the bass2jax/PJRT path), so the strictly-upper
triangle is never written. Row blocks are processed in DESCENDING order:
the wide stores go out first, keeping the DMA wire saturated while the
cheap remaining flag chunks compute.

Engine split: DVE computes the logits (one mul+reduce per row block) and
the small compare/fix-up ops, ACT does the Ln chain up front and half of
the per-row broadcasts (DVE the other half), GPSIMD does the triangle
trim via affine_select, and the two HWDGE rings (sync + scalar queues)
each stream one head's output stores. The PE is left empty on purpose —
its instruction-stream fetch at kernel start costs ~1us per IRAM block
and sits on the critical path.
"""

import numpy as np

B, S, D, T, H = 2, 2048, 128, 3, 8
P = 128           # partitions / row-block size
NB = S // P       # 16 row blocks
CB = 4            # row blocks per phase-A chunk
EPS = 1e-10
N_CORES = 8

_CACHE = {}


def _build_program():
    import concourse.bass as bass
    import concourse.bacc as bacc
    import concourse.tile as tile
    from concourse import mybir
    from contextlib import ExitStack

    f32 = mybir.dt.float32
    Alu = mybir.AluOpType
    Act = mybir.ActivationFunctionType

    nc = bacc.Bacc("TRN2", debug=False, num_devices=N_CORES)
    x_ap = nc.dram_tensor("x", [S, D], f32, kind="ExternalInput").ap()
    u_ap = nc.dram_tensor("u", [S, T], f32, kind="ExternalInput").ap()
    w_ap = nc.dram_tensor("w", [T, D], f32, kind="ExternalInput").ap()
    bb_ap = nc.dram_tensor("bb", [T], f32, kind="ExternalInput").ap()
    out_ap = nc.dram_tensor("out", [2, S, S], f32, kind="ExternalOutput").ap()

    # process row blocks in DESCENDING order: late blocks carry the widest
    # stores, so issuing them first keeps the DMA wire saturated while the
    # remaining (cheap) flag chunks compute; the first chunk is small to
    # minimize time-to-first-store.
    chunk_list = [(14, 2), (12, 2), (10, 2), (8, 2), (4, 4), (0, 4)]

    with tile.TileContext(nc) as tc, ExitStack() as ctx:
        singles = ctx.enter_context(tc.tile_pool(name="singles", bufs=1))
        outp = ctx.enter_context(tc.tile_pool(name="outp", bufs=NB))
        chunks = ctx.enter_context(tc.tile_pool(name="chunks", bufs=2))

        # --- input loads (sync/HWDGE queue — it is idle until stores begin) ---
        # W broadcast to every partition: wb[p, t, d] = W[t, d]; first in the
        # queue because the first logits mul is gated on its completion
        wb = singles.tile([P, T, D], f32)
        nc.sync.dma_start(out=wb, in_=bass.AP(w_ap.tensor, 0, [[0, P], [D, T], [1, D]]))
        # x_all[p, i, d] = x[128*i + p, d]; loaded in chunk order so the first
        # chunk's slice (and its completion semaphore) lands first
        x_all = singles.tile([P, NB, D], f32)
        u_all = singles.tile([P, NB, T], f32)
        for n, (c0, cb) in enumerate(chunk_list):
            nc.sync.dma_start(
                out=x_all[:, c0 : c0 + cb, :],
                in_=bass.AP(
                    x_ap.tensor, c0 * P * D, [[D, P], [P * D, cb], [1, D]]
                ),
            )
            if n == 1:
                # u_all[p, i, t] = u[128*i + p, t]
                nc.sync.dma_start(
                    out=u_all, in_=bass.AP(u_ap.tensor, 0, [[T, P], [P * T, NB], [1, T]])
                )

        # --- constants (gpsimd queue, off the critical store queue) ---
        # bias broadcast: bbb[p, t] = bias[t]
        bbb = singles.tile([P, T], f32)
        nc.gpsimd.dma_start(out=bbb, in_=bass.AP(bb_ap.tensor, 0, [[0, P], [1, T]]))
        zeros = singles.tile([P, S], f32)
        nc.vector.memset(zeros, 0.0)
        eps_t = singles.tile([P, 1], f32)
        nc.vector.memset(eps_t, EPS)
        ones128 = singles.tile([P, P], f32)
        nc.gpsimd.memset(ones128, 1.0)
        # diag128[p, j] = 1.0 if j == p else 0.0
        diag128 = singles.tile([P, P], f32)
        nc.gpsimd.affine_select(
            out=diag128, in_=ones128, pattern=[[-1, P]],
            compare_op=Alu.is_equal, fill=0.0, base=0, channel_multiplier=1,
        )

        nf = singles.tile([P, NB], f32)
        nfa = nf[:]
        ba = bbb[:]

        # gumbel for ALL blocks up front: g2 = ln(-ln(u + eps) + eps); done
        # once on ACT so no per-chunk critical chain crosses engines twice
        g1 = singles.tile([P, NB, T], f32)
        nc.scalar.activation(g1[:], u_all[:], Act.Ln, bias=eps_t[:, 0:1], scale=1.0)
        g2 = singles.tile([P, NB, T], f32)
        nc.scalar.activation(g2[:], g1[:], Act.Ln, bias=eps_t[:, 0:1], scale=-1.0)
        # fold the bias in once: z = logits - g2 + b == logits - (g2 - b)
        g2b = singles.tile([P, NB, T], f32)
        b_all = bass.AP(ba.tensor, ba.offset, [ba.ap[0], [0, NB], ba.ap[1]])
        nc.vector.tensor_sub(g2b[:], g2[:], b_all)

        for c0, cb in chunk_list:
            # --- phase A for blocks [c0, c0+cb): logits on DVE, one block per
            # mul/reduce so no single op clogs the DVE queue for long ---
            prod = chunks.tile([P, T, D], f32, tag="prod")
            logits = chunks.tile([P, CB, T], f32, tag="logits")
            wa = wb[:]
            for j in range(cb):
                # prod[p, t, d] = x_all[p, c0+j, d] * wb[p, t, d]
                xa = x_all[:, c0 + j, :]
                x_b = bass.AP(xa.tensor, xa.offset, [xa.ap[0], [0, T], xa.ap[1]])
                nc.vector.tensor_mul(prod[:], x_b, wa)
                nc.vector.reduce_sum(
                    logits[:, j, :], prod[:], axis=mybir.AxisListType.X
                )

            # z = logits + g + bias (bias pre-folded into g2b)
            z = chunks.tile([P, CB, T], f32, tag="z")
            nc.vector.tensor_sub(z[:, 0:cb], logits[:, 0:cb], g2b[:, c0 : c0 + cb, :])

            # notflag[p, i] = (z1 <= z0) | (z1 < z2)  == !(argmax picks index 1)
            za = z[:]

            def zcol(t):
                return bass.AP(za.tensor, za.offset + t, [za.ap[0], [T, cb]])

            c1 = chunks.tile([P, CB], f32, tag="c1")
            nc.vector.tensor_tensor(c1[:, 0:cb], zcol(1), zcol(0), op=Alu.is_le)
            c2 = chunks.tile([P, CB], f32, tag="c2")
            nc.vector.tensor_tensor(c2[:, 0:cb], zcol(1), zcol(2), op=Alu.is_lt)
            nc.vector.tensor_max(nfa[:, c0 : c0 + cb], c1[:, 0:cb], c2[:, 0:cb])

            # --- phase B for blocks [c0, c0+cb), widest stores first ---
            for i in range(c0 + cb - 1, c0 - 1, -1):
                r0 = P * i
                w_cols = r0 + P
                nf_col = nfa[:, i : i + 1]

                ot = outp.tile([P, w_cols], f32, tag=f"ot{i}", bufs=1)
                split = i >= 14  # first blocks out: overlap bcast with stores
                if r0 > 0 and split:
                    # first half of the prefix broadcast, shipped immediately
                    # so the wire starts while the second half builds
                    half = r0 // 2
                    nc.scalar.activation(
                        ot[:, 0:half], zeros[:, 0:half], Act.Identity,
                        bias=nf_col, scale=1.0,
                    )
                    nc.sync.dma_start(
                        out=out_ap[0, r0 : r0 + P, 0:half], in_=ot[:, 0:half]
                    )
                    nc.scalar.dma_start(
                        out=out_ap[1, r0 : r0 + P, 0:half], in_=ot[:, 0:half]
                    )
                    nc.scalar.activation(
                        ot[:, half:r0], zeros[:, half:r0], Act.Identity,
                        bias=nf_col, scale=1.0,
                    )
                elif r0 > 0:
                    # prefix cols [0, r0): notflag broadcast (ACT only — DVE
                    # broadcasts sit in the DVE queue between chunk A-chains
                    # and delay the next blocks' flags)
                    nc.scalar.activation(
                        ot[:, 0:r0], zeros[:, 0:r0], Act.Identity,
                        bias=nf_col, scale=1.0,
                    )
                # diag chunk: notflag broadcast, trimmed to lower triangle,
                # then the diagonal forced to 1 (covers tt==1 rows) — both on
                # GPSIMD so the store waits on a single engine chain
                nf_bcast = bass.AP(nfa.tensor, nfa.offset + i, [nfa.ap[0], [0, P]])
                nc.gpsimd.affine_select(
                    out=ot[:, r0:w_cols], in_=nf_bcast, pattern=[[-1, P]],
                    compare_op=Alu.is_ge, fill=0.0, base=0, channel_multiplier=1,
                )
                nc.gpsimd.affine_select(
                    out=ot[:, r0:w_cols], in_=ot[:, r0:w_cols], pattern=[[-1, P]],
                    compare_op=Alu.not_equal, fill=1.0, base=0, channel_multiplier=1,
                )

                # one head per HWDGE ring (sync + scalar) -> 2x the DMA lanes
                lo = r0 // 2 if split else 0
                nc.sync.dma_start(
                    out=out_ap[0, r0 : r0 + P, lo:w_cols], in_=ot[:, lo:w_cols]
                )
                nc.scalar.dma_start(
                    out=out_ap[1, r0 : r0 + P, lo:w_cols], in_=ot[:, lo:w_cols]
                )

    nc.compile()
    return nc


def _get_program():
    if "nc" not in _CACHE:
        _CACHE["nc"] = _build_program()
    return _CACHE["nc"]


def _make_in_maps(input_tensor, gumbel_u, W, b):
    x = np.ascontiguousarray(np.asarray(input_tensor, dtype=np.float32))
    u = np.ascontiguousarray(np.asarray(gumbel_u, dtype=np.float32))
    w = np.ascontiguousarray(np.asarray(W, dtype=np.float32))
    bb = np.ascontiguousarray(np.asarray(b, dtype=np.float32))
    in_maps = []
    for c in range(N_CORES):
        bi = c // (N_CORES // B)
        in_maps.append({"x": x[bi], "u": u[bi], "w": w, "bb": bb})
    return in_maps


def _assemble(results):
    full = np.empty((B, H, S, S), dtype=np.float32)
    for c in range(N_CORES):
        bi = c // (N_CORES // B)
        q = c % (N_CORES // B)
        full[bi, 2 * q] = results[c]["out"][0]
        full[bi, 2 * q + 1] = results[c]["out"][1]
    return full


def kernel(input_tensor, token_types, gumbel_u, W, b, **_ignored):
    from concourse.bass_utils import run_bass_kernel_spmd

    nc = _get_program()
    in_maps = _make_in_maps(input_tensor, gumbel_u, W, b)
    res = run_bass_kernel_spmd(nc, in_maps, core_ids=list(range(N_CORES)))
    return _assemble(res.results)
